# revision 11
# baseline (speedup 1.0000x reference)
"""Trainium2 Bass kernel for nn_DeltaSynapse.

I[b,o] = einsum('beo,dbe,deo,dbe->bo', Weff, Xd, delaymap, Wshort+1)
with Weff[b,e,o] = signs[e,o] * (W[e,o]*(1-frac[e,o]) + Wlong[b,e,o]*frac[e,o])
     signs[e,o] = sign(signs_pre[e]) * (W[e,o] > 0)

Rewrite: let G[d,b,e] = Xd*(Wshort+1), A = signs*W*(1-frac), SF = signs*frac.
  I[b,o] = sum_{d,e} G[d,b,e]*A[e,o]*dm[d,e,o]                    (term1: pure matmul)
         + sum_e Wlong[b,e,o] * H[b,e,o],                         (term2)
  H[b,e,o] = sum_d G[d,b,e]*Q[d,e,o],  Q = SF*dm.

Sharding: o (post) dim across 8 cores (No=256 each) -> every HBM byte read once.

Per core, per e-group g of J=16 e's (NG = N/J groups):
  - P = A_rep * dm_t, Q = SF_rep * dm_t   (DVE/GPSIMD tensor ops, bf16 2x)
  - term1 MM: I_psum[16,No] += gq1[:,g,:].T @ P        (K=(d,j)=128, M=B)
  - H' MM per b-half: Hp[128,No] = gblk_half.T @ Q     (block-diag stationary:
        lhsT[(d,j),(bb,j')] = G[d,b,g*16+j] * delta_{j,j'} -> M=(bb,j')=128 full)
  - Z = wl_t * Hp  (after ACT evac PSUM->SBUF)
  - Zred MM: I_psum += eh[h].T @ Z  (0/1 indicator columns sum j' per b)
"""

import os
import sys
import numpy as np

sys.path.insert(0, "/opt/trn_rl_repo")

import ml_dtypes

BF16 = ml_dtypes.bfloat16

# problem constants
D, B, N = 8, 16, 2048
NCORES = 8
NO = N // NCORES  # per-core o-slice width
J = 16            # e's per group
NG = N // J       # e-groups per core
ET = N // 128     # e-tiles (A/SF sbuf layout)
HB = B // 2       # b per half


def _consts():
    """Constant matrices (same all cores)."""
    # diag mask delta_{j,j'} tiled over (half, bb): [128, 256]
    p = np.arange(128)
    m = np.arange(256)
    dmask = (p[:, None] % J == m[None, :] % J).astype(np.float32)
    # Zred indicator eh[h][(bb,j'), b'] = 1 iff b' == h*8+bb
    eh = np.zeros((2, 128, B), dtype=np.float32)
    for h in range(2):
        for bb in range(HB):
            eh[h, bb * J:(bb + 1) * J, h * HB + bb] = 1.0
    return dmask, eh


def host_prep(W, Wlong, Wshort, Xd, delaymap, STDP_frac, signs_pre, use_bf16=True):
    """Host-side input prep: signs/A/SF fusion, G layout transforms, o-shard."""
    dt = BF16 if use_bf16 else np.float32
    W = np.asarray(W, np.float32)
    frac = np.asarray(STDP_frac, np.float32)
    signs = np.where(W > 0, np.sign(np.asarray(signs_pre, np.float32))[:, None],
                     np.float32(0.0))
    A = (signs * W * (1.0 - frac)).astype(np.float32)
    SF = (signs * frac).astype(np.float32)
    G = (np.asarray(Xd, np.float32) *
         (np.asarray(Wshort, np.float32) + 1.0))  # [D,B,N]

    # gq1[p=(d,j), g, b] = G[d, b, g*J+j]
    # G[d,b,e] -> [d, b, g, j] -> (d j) g b
    Gr = G.reshape(D, B, NG, J)
    gq1 = np.ascontiguousarray(Gr.transpose(0, 3, 2, 1).reshape(D * J, NG, B)).astype(dt)

    # gblk[g, p=(d,j), h*128 + bb*16 + j'] = G[d, h*8+bb, g*J+j] * delta_{j,j'}
    # build from gq1 fp32 for exactness of zeros
    gq1f = Gr.transpose(0, 3, 2, 1).reshape(128, NG, B)  # fp32
    dmask, eh = _consts()
    # gblk[g][p, (h,bb,j')] = gq1f[p, g, h*8+bb] * dmask[p, (h,bb,j')]
    gb = gq1f.transpose(1, 0, 2)  # [NG, 128, B]
    gb = gb.reshape(NG, 128, 2, HB)[:, :, :, :, None] * np.ones((1, 1, 1, 1, J), np.float32)
    gb = gb.reshape(NG, 128, 256) * dmask[None]
    gblk = np.ascontiguousarray(gb).astype(dt)

    def shard_eo(M_, c):
        # natural [N, No] per-core slice
        return np.ascontiguousarray(M_[:, c * NO:(c + 1) * NO]).astype(dt)

    ins = []
    dmf = np.asarray(delaymap, np.float32)
    wlf = np.asarray(Wlong, np.float32)
    for c in range(NCORES):
        sl = slice(c * NO, (c + 1) * NO)
        ins.append({
            "dm": np.ascontiguousarray(dmf[:, :, sl]).astype(dt),
            "wl": np.ascontiguousarray(wlf[:, :, sl]).astype(dt),
            "Amat": shard_eo(A, c),
            "SFmat": shard_eo(SF, c),
            "gq1": gq1,
            "gblk": gblk,
        })
    return ins


def build_nc(use_bf16=True, n_cores=NCORES, no=NO, ng=NG):
    """Build the SPMD Bass program (same on all cores)."""
    import concourse.bass as bass
    import concourse.bacc as bacc
    import concourse.mybir as mybir
    import concourse.tile as tile
    from contextlib import ExitStack

    dt_big = mybir.dt.bfloat16 if use_bf16 else mybir.dt.float32
    f32 = mybir.dt.float32
    n = ng * J
    et = n // 128

    nc = bacc.Bacc("TRN2", target_bir_lowering=False, debug=False,
                   num_devices=n_cores)

    dm = nc.declare_dram_parameter("dm", [D, n, no], dt_big, isOutput=False).ap()
    wl = nc.declare_dram_parameter("wl", [B, n, no], dt_big, isOutput=False).ap()
    Amat = nc.declare_dram_parameter("Amat", [n, no], dt_big, isOutput=False).ap()
    SFmat = nc.declare_dram_parameter("SFmat", [n, no], dt_big, isOutput=False).ap()
    gq1 = nc.declare_dram_parameter("gq1", [128, ng, B], dt_big, isOutput=False).ap()
    gblk = nc.declare_dram_parameter("gblk", [ng, 128, 256], dt_big, isOutput=False).ap()
    out = nc.declare_dram_parameter("out", [B, no], f32, isOutput=True).ap()

    dmask_np, eh_np = _consts()
    np_dt = BF16 if use_bf16 else np.float32
    eh_dram = nc.inline_tensor(eh_np.astype(np_dt), name="ehc")

    def mmdt(ap):
        # PE streams f32 data as float32r (full rate at free>=256)
        return ap if use_bf16 else ap.bitcast(mybir.dt.float32r)

    with tile.TileContext(nc) as tc, ExitStack() as ctx:
        res = ctx.enter_context(tc.tile_pool(name="res", bufs=1))
        # resident tensors
        gq1_sb = res.tile([128, ng, B], dt_big)
        eh_sb = res.tile([128, 2, B], dt_big)
        nc.gpsimd.dma_start(out=eh_sb[:, :, :], in_=eh_dram.ap().rearrange("h p b -> p h b"))
        nc.gpsimd.dma_start(out=gq1_sb[:, :, :], in_=gq1)

        dm_pool = ctx.enter_context(tc.tile_pool(name="dmp", bufs=4))
        gb_pool = ctx.enter_context(tc.tile_pool(name="gbp", bufs=4))
        wl_pool = ctx.enter_context(tc.tile_pool(name="wlp", bufs=6))
        rep_pool = ctx.enter_context(tc.tile_pool(name="repp", bufs=6))
        pq_pool = ctx.enter_context(tc.tile_pool(name="pqp", bufs=6))
        hz_pool = ctx.enter_context(tc.tile_pool(name="hzp", bufs=6))
        psum_h = ctx.enter_context(tc.tile_pool(name="psh", bufs=4, space="PSUM"))
        psum_i = ctx.enter_context(tc.tile_pool(name="psi", bufs=1, space="PSUM"))
        out_pool = ctx.enter_context(tc.tile_pool(name="outp", bufs=1))

        I_psum = psum_i.tile([B, no], f32)

        for g in range(ng):
            dm_t = dm_pool.tile([128, no], dt_big, tag="dm")
            nc.gpsimd.dma_start(out=dm_t[:, :], in_=dm[:, g * J:(g + 1) * J, :])
            gb_t = gb_pool.tile([128, 256], dt_big, tag="gb")
            nc.gpsimd.dma_start(out=gb_t[:, :], in_=gblk[g])

            # replicate A/SF 16-row slice across the 8 d-blocks (DRAM broadcast read)
            arep = rep_pool.tile([128, no], dt_big, tag="ar")
            srep = rep_pool.tile([128, no], dt_big, tag="sr")
            src_a = Amat[g * J:(g + 1) * J, :].unsqueeze(0).broadcast_to((D, J, no))
            src_s = SFmat[g * J:(g + 1) * J, :].unsqueeze(0).broadcast_to((D, J, no))
            nc.gpsimd.dma_start(out=arep[:, :], in_=src_a)
            nc.gpsimd.dma_start(out=srep[:, :], in_=src_s)

            P_t = pq_pool.tile([128, no], dt_big, tag="P")
            Q_t = pq_pool.tile([128, no], dt_big, tag="Q")
            nc.vector.tensor_mul(P_t[:, :], dm_t[:, :], arep[:, :])
            nc.gpsimd.tensor_mul(Q_t[:, :], dm_t[:, :], srep[:, :])

            # term1
            nc.tensor.matmul(I_psum[:, :], mmdt(gq1_sb[:, g, :]), mmdt(P_t[:, :]),
                             start=(g == 0), stop=False)

            for h in range(2):
                Hp = psum_h.tile([128, no], f32, tag="hp")
                nc.tensor.matmul(Hp[:, :], mmdt(gb_t[:, h * 128:(h + 1) * 128]),
                                 mmdt(Q_t[:, :]), start=True, stop=True)
                wl_t = wl_pool.tile([128, no], dt_big, tag="wl")
                nc.gpsimd.dma_start(out=wl_t[:, :],
                                  in_=wl[h * HB:(h + 1) * HB, g * J:(g + 1) * J, :])
                Hs = hz_pool.tile([128, no], dt_big, tag="hs")
                nc.scalar.copy(Hs[:, :], Hp[:, :])
                Z_t = hz_pool.tile([128, no], dt_big, tag="z")
                nc.vector.tensor_mul(Z_t[:, :], wl_t[:, :], Hs[:, :])
                last = (g == ng - 1) and (h == 1)
                nc.tensor.matmul(I_psum[:, :], mmdt(eh_sb[:, h, :]), mmdt(Z_t[:, :]),
                                 start=False, stop=last)

        I_sb = out_pool.tile([B, no], f32)
        nc.scalar.copy(I_sb[:, :], I_psum[:, :])
        nc.gpsimd.dma_start(out=out, in_=I_sb[:, :])

    nc.compile()
    return nc


_CACHE = {}


def kernel(W, Wlong, Wshort, Xd, delaymap, STDP_frac, signs_pre):
    from concourse.bass_utils import run_bass_kernel_spmd

    use_bf16 = os.environ.get("DS_FP32", "0") != "1"
    ins = host_prep(W, Wlong, Wshort, Xd, delaymap, STDP_frac, signs_pre, use_bf16)
    key = ("nc", use_bf16)
    if key not in _CACHE:
        _CACHE[key] = build_nc(use_bf16)
    nc = _CACHE[key]
    r = run_bass_kernel_spmd(nc, ins, list(range(NCORES)))
    outs = [r.results[c]["out"] for c in range(NCORES)]
    return np.concatenate(outs, axis=1).astype(np.float32)


if __name__ == "__main__":
    pass
